# revision 20
# baseline (speedup 1.0000x reference)
"""Pairwise-distance adjacency kernel (exp(-||a-b||)) for Trainium2, 8 cores.

Problem: inputs1 [4,4096,256], inputs2 [4,4096,256] (fp32)
         out[b,n,m] = exp(-sqrt(clip(||a_bn||^2 - 2 a.b + ||b_bm||^2)))

Sharding: 8 shards = (batch b in 0..3) x (row-half h in 0..1) of inputs1.
Each core computes a [2048, 4096] block of the output for one batch.

Per-core pipeline (v3):
  - host ships K-major aT [256,2048], bT [256,4096] (fp32r matmul operands)
    and row-major a [2048,256] (for the na reduction layout)
  - na[m] per-partition (fp32): VectorE square + innermost-axis reduce
  - nb[n]/2 via f32r ones-matmul -> fp32 row; replicated to all partitions
    with K=1 outer-product matmuls using an fp32r hi/lo pair (full fp32
    precision despite the fp32r operand format)
  - main loop per [128,2048] psum tile: 8 fp32r matmuls -> psum = a.b
  - VectorE: u = psum - nb/2              (PSUM -> SBUF staging)
  - ScalarE pass 1: D = Sqrt(-2*u + na)   (in place, [128,4096] per m-tile)
  - ScalarE pass 2: out = Exp(-D)         (in place; sqrt/exp table switches
    batched in groups of G row-tiles, enforced with explicit deps)
  - DMA staging -> DRAM
"""

import os
import sys

for _p in ("/opt/trn_rl_repo", "/root/.axon_site/_ro/trn_rl_repo"):
    if os.path.isdir(_p) and _p not in sys.path:
        sys.path.append(_p)

import numpy as np

import concourse.bass as bass
import concourse.mybir as mybir
from concourse import bacc
from concourse.tile import TileContext, add_dep_helper
from concourse.bass_utils import run_bass_kernel_spmd

F32 = mybir.dt.float32
F32R = mybir.dt.float32r
U32 = mybir.dt.uint32
AL = mybir.AluOpType
AF = mybir.ActivationFunctionType
AX = mybir.AxisListType

P = 128          # partitions
D = 256          # feature dim (contraction)
KS = D // P      # 2 K-subtiles
M = 2048         # rows per core (inputs1 shard)
N = 4096         # cols per core (full inputs2 rows for one batch)
MT = M // P      # 16 m-tiles
NCH = 512        # matmul free-dim chunk (one PSUM bank)
PSW = 2048       # psum tile width (4 banks); 2 tiles = all 8 banks
G = 8            # row-tiles per ACT table-set group
SBUFS = G      # staging buffers (SBUF-limited; no spare fits)

B_FULL, N_FULL = 4, 4096
N_CORES = 8

_nc_cache = None


def _raw(inst):
    return getattr(inst, "ins", inst)


def _build():
    """Build the single-core Bass program (identical on all 8 cores)."""
    nc = bacc.Bacc()
    aT_d = nc.declare_dram_parameter("aT", [D, M], F32R, isOutput=False)
    bT_d = nc.declare_dram_parameter("bT", [D, N], F32R, isOutput=False)
    a_d = nc.declare_dram_parameter("a", [M, D], F32, isOutput=False)
    out_d = nc.declare_dram_parameter("o", [M, N], F32, isOutput=True)
    dbg_d = nc.declare_dram_parameter("dbg", [P, N], F32, isOutput=True)

    out_r = out_d[:, :].rearrange("(t p) n -> t p n", p=P)

    with TileContext(nc) as tc:
        with (
            tc.tile_pool(name="const", bufs=1) as const,
            tc.tile_pool(name="psum", bufs=2, space="PSUM") as psum,
        ):
            aT_r = const.tile([P, KS, M], F32R)
            bT_r = const.tile([P, KS, N], F32R)
            na_pm = const.tile([P, MT], F32)      # per-partition na bias
            nbh_repl = const.tile([P, N], F32)    # nb/2 on every partition
            ones2 = const.tile([P, 2], F32R)      # norm-reduce lhsT
            ones_row = const.tile([P, P], F32R)   # outer-product lhsT [1,128]

            # load order matters: bT feeds the longest preamble chain
            nc.sync.dma_start(
                out=bT_r[:], in_=bT_d[:, :].rearrange("(ks p) n -> p ks n", p=P))
            nc.vector.memset(ones2[:, :].bitcast(U32), 0x3F800000)
            nc.vector.memset(ones_row[0:1, :].bitcast(U32), 0x3F800000)

            with tc.tile_pool(name="tmp", bufs=1) as tmp:
                a_rm = tmp.tile([P, MT, D], F32, tag="arm")
                nc.sync.dma_start(
                    out=a_rm[:], in_=a_d[:, :].rearrange("(t p) k -> p t k", p=P))
                nc.sync.dma_start(
                    out=aT_r[:], in_=aT_d[:, :].rearrange("(ks p) m -> p ks m", p=P))

                # --- na (per-partition, fp32): square + innermost reduce
                asq = tmp.tile([P, MT, D], F32, tag="asq")
                nc.vector.tensor_tensor(
                    out=asq[:], in0=a_rm[:], in1=a_rm[:], op=AL.mult)
                nc.vector.reduce_sum(na_pm[:, :, None], asq[:], axis=AX.X)

                # --- nb/2 (free-major row) via f32r ones-matmul
                bsq = tmp.tile([P, KS, N], F32R, tag="bsq")
                nc.vector.tensor_tensor(
                    out=bsq[:], in0=bT_r[:], in1=bT_r[:], op=AL.mult)
                nbh_row = tmp.tile([P, N], F32, tag="nbrow")
                nbh_hi = tmp.tile([P, N], F32R, tag="nbhi")
                nbh_lo = tmp.tile([P, N], F32R, tag="nblo")
                nbh_lof = tmp.tile([P, N], F32, tag="nblof")
                for half in range(N // PSW):
                    hsl = slice(half * PSW, (half + 1) * PSW)
                    pnb = psum.tile([P, PSW], F32, tag="pt")
                    for c in range(PSW // NCH):
                        n0 = half * PSW + c * NCH
                        for k in range(KS):
                            nc.tensor.matmul(
                                pnb[0:2, c * NCH:(c + 1) * NCH],
                                lhsT=ones2[:, :],
                                rhs=bsq[:, k, n0:n0 + NCH],
                                start=(k == 0),
                                stop=(k == KS - 1),
                            )
                    nc.vector.tensor_scalar_mul(nbh_row[0:1, hsl],
                                                pnb[0:1, :], 0.5)
                # fp32r hi/lo pair: hi = round(nb/2), lo = round(nb/2 - hi)
                nc.vector.tensor_copy(out=nbh_hi[0:1, :], in_=nbh_row[0:1, :])
                nc.vector.tensor_tensor(out=nbh_lof[0:1, :],
                                        in0=nbh_row[0:1, :],
                                        in1=nbh_hi[0:1, :], op=AL.subtract)
                nc.vector.tensor_copy(out=nbh_lo[0:1, :], in_=nbh_lof[0:1, :])

                # replicate across partitions: ones[1,128] (x) (hi + lo)
                for c in range(N // NCH):
                    prep = psum.tile([P, PSW], F32, tag="pt")
                    sl = slice(c * NCH, (c + 1) * NCH)
                    nc.tensor.matmul(prep[:, 0:NCH], lhsT=ones_row[0:1, :],
                                     rhs=nbh_hi[0:1, sl], start=True, stop=False)
                    nc.tensor.matmul(prep[:, 0:NCH], lhsT=ones_row[0:1, :],
                                     rhs=nbh_lo[0:1, sl], start=False, stop=True)
                    nc.vector.tensor_copy(out=nbh_repl[:, sl],
                                          in_=prep[:, 0:NCH])

            # --- main loop ---
            with tc.tile_pool(name="stage", bufs=SBUFS) as stage_pool:
                staged = []
                last_exp = [None]

                def flush():
                    first_exp = None
                    for st_, i_, last_sqrt in staged:
                        e = nc.scalar.activation(
                            out=st_[:], in_=st_[:], func=AF.Exp,
                            bias=0.0, scale=-1.0,
                        )
                        if first_exp is None:
                            first_exp = e
                            # no exp before the group's last sqrt
                            add_dep_helper(_raw(e), _raw(last_sqrt),
                                           reason="act-table group: exp after sqrt")
                        last_exp[0] = e
                        nc.sync.dma_start(out=out_r[i_], in_=st_[:])
                    staged.clear()

                for i in range(MT):
                    st = stage_pool.tile([P, N], F32, tag="stage")
                    m0 = i * P
                    for half in range(N // PSW):
                        hsl = slice(half * PSW, (half + 1) * PSW)
                        pt = psum.tile([P, PSW], F32, tag="pt")
                        for c in range(PSW // NCH):
                            n0 = half * PSW + c * NCH
                            ps = pt[:, c * NCH:(c + 1) * NCH]
                            for k in range(KS):
                                nc.tensor.matmul(
                                    ps,
                                    lhsT=aT_r[:, k, m0:m0 + P],
                                    rhs=bT_r[:, k, n0:n0 + NCH],
                                    start=(k == 0),
                                    stop=(k == KS - 1),
                                )
                        # u = a.b - nb/2  (PSUM -> SBUF staging)
                        nc.vector.tensor_tensor(
                            out=st[:, hsl], in0=pt[:], in1=nbh_repl[:, hsl],
                            op=AL.subtract,
                        )
                    # D = sqrt(-2*u + na), whole m-tile at once
                    s = nc.scalar.activation(
                        out=st[:], in_=st[:], func=AF.Sqrt,
                        bias=na_pm[:, i:i + 1], scale=-2.0,
                    )
                    if last_exp[0] is not None:
                        # no sqrt of this group before last group's exps
                        add_dep_helper(_raw(s), _raw(last_exp[0]),
                                       reason="act-table group: sqrt after exp")
                        last_exp[0] = None
                    if i == 0:
                        # debug tap: D values of the first row-tile
                        nc.sync.dma_start(out=dbg_d[:, :], in_=st[:])
                    staged.append((st, i, s))
                    if len(staged) == G or i == MT - 1:
                        flush()

    nc.compile()
    return nc


def _get_nc():
    global _nc_cache
    if _nc_cache is None:
        _nc_cache = _build()
    return _nc_cache


def _make_in_maps(inputs1, inputs2):
    inputs1 = np.asarray(inputs1, dtype=np.float32)
    inputs2 = np.asarray(inputs2, dtype=np.float32)
    in_maps = []
    for c in range(N_CORES):
        b, h = divmod(c, 2)
        a = inputs1[b, h * M:(h + 1) * M, :]
        in_maps.append({
            "aT": np.ascontiguousarray(a.T),
            "bT": np.ascontiguousarray(inputs2[b].T),
            "a": np.ascontiguousarray(a),
        })
    return in_maps


def _run_spmd(inputs1, inputs2, trace=False):
    nc = _get_nc()
    in_maps = _make_in_maps(inputs1, inputs2)
    return run_bass_kernel_spmd(nc, in_maps, core_ids=list(range(N_CORES)),
                                trace=trace)


def _assemble(results):
    out = np.empty((B_FULL, 2 * M, N_FULL), np.float32)
    for c in range(N_CORES):
        b, h = divmod(c, 2)
        out[b, h * M:(h + 1) * M, :] = results[c]["o"]
    return out


def kernel(inputs1, inputs2):
    res = _run_spmd(inputs1, inputs2, trace=False)
    return _assemble(res.results)


# revision 22
# speedup vs baseline: 1.0001x; 1.0001x over previous
"""Pairwise-distance adjacency kernel (exp(-||a-b||)) for Trainium2, 8 cores.

Problem: inputs1 [4,4096,256], inputs2 [4,4096,256] (fp32)
         out[b,n,m] = exp(-sqrt(clip(||a_bn||^2 - 2 a.b + ||b_bm||^2)))

Sharding: 8 shards = (batch b in 0..3) x (row-half h in 0..1) of inputs1.
Each core computes a [2048, 4096] block of the output for one batch.

Per-core pipeline (v3):
  - host ships K-major aT [256,2048], bT [256,4096] (fp32r matmul operands)
    and row-major a [2048,256] (for the na reduction layout)
  - na[m] per-partition (fp32): VectorE square + innermost-axis reduce
  - nb[n]/2 via f32r ones-matmul -> fp32 row; replicated to all partitions
    with K=1 outer-product matmuls using an fp32r hi/lo pair (full fp32
    precision despite the fp32r operand format)
  - main loop per [128,2048] psum tile: 8 fp32r matmuls -> psum = a.b
  - VectorE: u = psum - nb/2              (PSUM -> SBUF staging)
  - ScalarE pass 1: D = Sqrt(-2*u + na)   (in place, [128,4096] per m-tile)
  - ScalarE pass 2: out = Exp(-D)         (in place; sqrt/exp table switches
    batched in groups of G row-tiles, enforced with explicit deps)
  - DMA staging -> DRAM
"""

import os
import sys

for _p in ("/opt/trn_rl_repo", "/root/.axon_site/_ro/trn_rl_repo"):
    if os.path.isdir(_p) and _p not in sys.path:
        sys.path.append(_p)

import numpy as np

import concourse.bass as bass
import concourse.mybir as mybir
from concourse import bacc
from concourse.tile import TileContext, add_dep_helper
from concourse.bass_utils import run_bass_kernel_spmd

F32 = mybir.dt.float32
F32R = mybir.dt.float32r
U32 = mybir.dt.uint32
AL = mybir.AluOpType
AF = mybir.ActivationFunctionType
AX = mybir.AxisListType

P = 128          # partitions
D = 256          # feature dim (contraction)
KS = D // P      # 2 K-subtiles
M = 2048         # rows per core (inputs1 shard)
N = 4096         # cols per core (full inputs2 rows for one batch)
MT = M // P      # 16 m-tiles
NCH = 512        # matmul free-dim chunk (one PSUM bank)
PSW = 2048       # psum tile width (4 banks); 2 tiles = all 8 banks
# ACT table-set groups in half-tile units ([128,2048] staging buffers).
# Uneven groups: big first groups amortize table loads, small last group
# shrinks the serial exp+DMA tail.
GROUP_ENDS = (15, 27, 31)   # inclusive last half-tile index of each group
SBUFS = 17                  # 16 halves per max group + 1 spare

B_FULL, N_FULL = 4, 4096
N_CORES = 8

_nc_cache = None


def _raw(inst):
    return getattr(inst, "ins", inst)


def _build():
    """Build the single-core Bass program (identical on all 8 cores)."""
    nc = bacc.Bacc()
    aT_d = nc.declare_dram_parameter("aT", [D, M], F32R, isOutput=False)
    bT_d = nc.declare_dram_parameter("bT", [D, N], F32R, isOutput=False)
    a_d = nc.declare_dram_parameter("a", [M, D], F32, isOutput=False)
    out_d = nc.declare_dram_parameter("o", [M, N], F32, isOutput=True)
    dbg_d = nc.declare_dram_parameter("dbg", [P, N], F32, isOutput=True)

    out_r = out_d[:, :].rearrange("(t p) n -> t p n", p=P)

    with TileContext(nc) as tc:
        with (
            tc.tile_pool(name="const", bufs=1) as const,
            tc.tile_pool(name="psum", bufs=2, space="PSUM") as psum,
        ):
            aT_r = const.tile([P, KS, M], F32R)
            bT_r = const.tile([P, KS, N], F32R)
            na_pm = const.tile([P, MT], F32)      # per-partition na bias
            nbh_repl = const.tile([P, N], F32)    # nb/2 on every partition
            ones2 = const.tile([P, 2], F32R)      # norm-reduce lhsT
            ones_row = const.tile([P, P], F32R)   # outer-product lhsT [1,128]

            # load order matters: bT feeds the longest preamble chain
            nc.sync.dma_start(
                out=bT_r[:], in_=bT_d[:, :].rearrange("(ks p) n -> p ks n", p=P))
            nc.vector.memset(ones2[:, :].bitcast(U32), 0x3F800000)
            nc.vector.memset(ones_row[0:1, :].bitcast(U32), 0x3F800000)

            with tc.tile_pool(name="tmp", bufs=1) as tmp:
                a_rm = tmp.tile([P, MT, D], F32, tag="arm")
                nc.sync.dma_start(
                    out=a_rm[:], in_=a_d[:, :].rearrange("(t p) k -> p t k", p=P))
                nc.sync.dma_start(
                    out=aT_r[:], in_=aT_d[:, :].rearrange("(ks p) m -> p ks m", p=P))

                # --- na (per-partition, fp32): square + innermost reduce
                asq = tmp.tile([P, MT, D], F32, tag="asq")
                nc.vector.tensor_tensor(
                    out=asq[:], in0=a_rm[:], in1=a_rm[:], op=AL.mult)
                nc.vector.reduce_sum(na_pm[:, :, None], asq[:], axis=AX.X)

                # --- nb/2 (free-major row) via f32r ones-matmul
                bsq = tmp.tile([P, KS, N], F32R, tag="bsq")
                nc.vector.tensor_tensor(
                    out=bsq[:], in0=bT_r[:], in1=bT_r[:], op=AL.mult)
                nbh_row = tmp.tile([P, N], F32, tag="nbrow")
                nbh_hi = tmp.tile([P, N], F32R, tag="nbhi")
                nbh_lo = tmp.tile([P, N], F32R, tag="nblo")
                nbh_lof = tmp.tile([P, N], F32, tag="nblof")
                for half in range(N // PSW):
                    hsl = slice(half * PSW, (half + 1) * PSW)
                    pnb = psum.tile([P, PSW], F32, tag="pt")
                    for c in range(PSW // NCH):
                        n0 = half * PSW + c * NCH
                        for k in range(KS):
                            nc.tensor.matmul(
                                pnb[0:2, c * NCH:(c + 1) * NCH],
                                lhsT=ones2[:, :],
                                rhs=bsq[:, k, n0:n0 + NCH],
                                start=(k == 0),
                                stop=(k == KS - 1),
                            )
                    nc.vector.tensor_scalar_mul(nbh_row[0:1, hsl],
                                                pnb[0:1, :], 0.5)
                # fp32r hi/lo pair: hi = round(nb/2), lo = round(nb/2 - hi)
                nc.vector.tensor_copy(out=nbh_hi[0:1, :], in_=nbh_row[0:1, :])
                nc.vector.tensor_tensor(out=nbh_lof[0:1, :],
                                        in0=nbh_row[0:1, :],
                                        in1=nbh_hi[0:1, :], op=AL.subtract)
                nc.vector.tensor_copy(out=nbh_lo[0:1, :], in_=nbh_lof[0:1, :])

                # replicate across partitions: ones[1,128] (x) (hi + lo)
                for c in range(N // NCH):
                    prep = psum.tile([P, PSW], F32, tag="pt")
                    sl = slice(c * NCH, (c + 1) * NCH)
                    nc.tensor.matmul(prep[:, 0:NCH], lhsT=ones_row[0:1, :],
                                     rhs=nbh_hi[0:1, sl], start=True, stop=False)
                    nc.tensor.matmul(prep[:, 0:NCH], lhsT=ones_row[0:1, :],
                                     rhs=nbh_lo[0:1, sl], start=False, stop=True)
                    nc.vector.tensor_copy(out=nbh_repl[:, sl],
                                          in_=prep[:, 0:NCH])

            # --- main loop (half-tile units: u = (i, half)) ---
            with tc.tile_pool(name="stage", bufs=SBUFS) as stage_pool:
                staged = []
                last_exp = [None]

                def flush():
                    first_exp = None
                    for st_, i_, half_, last_sqrt in staged:
                        e = nc.scalar.activation(
                            out=st_[:], in_=st_[:], func=AF.Exp,
                            bias=0.0, scale=-1.0,
                        )
                        if first_exp is None:
                            first_exp = e
                            # no exp before the group's last sqrt
                            add_dep_helper(_raw(e), _raw(last_sqrt),
                                           reason="act-table group: exp after sqrt")
                        last_exp[0] = e
                        nc.sync.dma_start(
                            out=out_r[i_, :, half_ * PSW:(half_ + 1) * PSW],
                            in_=st_[:])
                    staged.clear()

                for u in range(MT * 2):
                    i, half = divmod(u, 2)
                    st = stage_pool.tile([P, PSW], F32, tag="stage")
                    m0 = i * P
                    pt = psum.tile([P, PSW], F32, tag="pt")
                    for c in range(PSW // NCH):
                        n0 = half * PSW + c * NCH
                        ps = pt[:, c * NCH:(c + 1) * NCH]
                        for k in range(KS):
                            nc.tensor.matmul(
                                ps,
                                lhsT=aT_r[:, k, m0:m0 + P],
                                rhs=bT_r[:, k, n0:n0 + NCH],
                                start=(k == 0),
                                stop=(k == KS - 1),
                            )
                    # u = a.b - nb/2  (PSUM -> SBUF staging)
                    nc.vector.tensor_tensor(
                        out=st[:], in0=pt[:],
                        in1=nbh_repl[:, half * PSW:(half + 1) * PSW],
                        op=AL.subtract,
                    )
                    # D = sqrt(-2*u + na)
                    s = nc.scalar.activation(
                        out=st[:], in_=st[:], func=AF.Sqrt,
                        bias=na_pm[:, i:i + 1], scale=-2.0,
                    )
                    if last_exp[0] is not None:
                        # no sqrt of this group before last group's exps
                        add_dep_helper(_raw(s), _raw(last_exp[0]),
                                       reason="act-table group: sqrt after exp")
                        last_exp[0] = None
                    if i == 0:
                        # debug tap: D values of the first row-tile
                        nc.sync.dma_start(
                            out=dbg_d[:, half * PSW:(half + 1) * PSW],
                            in_=st[:])
                    staged.append((st, i, half, s))
                    if u in GROUP_ENDS:
                        flush()

    nc.compile()
    return nc


def _get_nc():
    global _nc_cache
    if _nc_cache is None:
        _nc_cache = _build()
    return _nc_cache


def _make_in_maps(inputs1, inputs2):
    inputs1 = np.asarray(inputs1, dtype=np.float32)
    inputs2 = np.asarray(inputs2, dtype=np.float32)
    in_maps = []
    for c in range(N_CORES):
        b, h = divmod(c, 2)
        a = inputs1[b, h * M:(h + 1) * M, :]
        in_maps.append({
            "aT": np.ascontiguousarray(a.T),
            "bT": np.ascontiguousarray(inputs2[b].T),
            "a": np.ascontiguousarray(a),
        })
    return in_maps


def _run_spmd(inputs1, inputs2, trace=False):
    nc = _get_nc()
    in_maps = _make_in_maps(inputs1, inputs2)
    return run_bass_kernel_spmd(nc, in_maps, core_ids=list(range(N_CORES)),
                                trace=trace)


def _assemble(results):
    out = np.empty((B_FULL, 2 * M, N_FULL), np.float32)
    for c in range(N_CORES):
        b, h = divmod(c, 2)
        out[b, h * M:(h + 1) * M, :] = results[c]["o"]
    return out


def kernel(inputs1, inputs2):
    res = _run_spmd(inputs1, inputs2, trace=False)
    return _assemble(res.results)


# revision 23
# speedup vs baseline: 1.0773x; 1.0772x over previous
"""Pairwise-distance adjacency kernel (exp(-||a-b||)) for Trainium2, 8 cores.

Problem: inputs1 [4,4096,256], inputs2 [4,4096,256] (fp32)
         out[b,n,m] = exp(-sqrt(clip(||a_bn||^2 - 2 a.b + ||b_bm||^2)))

Sharding: 8 shards = (batch b in 0..3) x (row-half h in 0..1) of inputs1.
Each core computes a [2048, 4096] block of the output for one batch.

Per-core pipeline (v3):
  - host ships K-major aT [256,2048], bT [256,4096] (fp32r matmul operands)
    and row-major a [2048,256] (for the na reduction layout)
  - na[m] per-partition (fp32): VectorE square + innermost-axis reduce
  - nb[n]/2 via f32r ones-matmul -> fp32 row; replicated to all partitions
    with K=1 outer-product matmuls using an fp32r hi/lo pair (full fp32
    precision despite the fp32r operand format)
  - main loop per [128,2048] psum tile: 8 fp32r matmuls -> psum = a.b
  - VectorE: u = psum - nb/2              (PSUM -> SBUF staging)
  - ScalarE pass 1: D = Sqrt(-2*u + na)   (in place, [128,4096] per m-tile)
  - ScalarE pass 2: out = Exp(-D)         (in place; sqrt/exp table switches
    batched in groups of G row-tiles, enforced with explicit deps)
  - DMA staging -> DRAM
"""

import os
import sys

for _p in ("/opt/trn_rl_repo", "/root/.axon_site/_ro/trn_rl_repo"):
    if os.path.isdir(_p) and _p not in sys.path:
        sys.path.append(_p)

import numpy as np

import concourse.bass as bass
import concourse.mybir as mybir
from concourse import bacc
from concourse.tile import TileContext, add_dep_helper
from concourse.bass_utils import run_bass_kernel_spmd

F32 = mybir.dt.float32
F32R = mybir.dt.float32r
U32 = mybir.dt.uint32
AL = mybir.AluOpType
AF = mybir.ActivationFunctionType
AX = mybir.AxisListType

P = 128          # partitions
D = 256          # feature dim (contraction)
KS = D // P      # 2 K-subtiles
M = 2048         # rows per core (inputs1 shard)
N = 4096         # cols per core (full inputs2 rows for one batch)
MT = M // P      # 16 m-tiles
NCH = 512        # matmul free-dim chunk (one PSUM bank)
PSW = 2048       # psum tile width (4 banks); 2 tiles = all 8 banks
# ACT table-set groups in half-tile units ([128,2048] staging buffers).
# Uneven groups: big first groups amortize table loads, small last group
# shrinks the serial exp+DMA tail.
GROUP_ENDS = (15, 27, 31)   # inclusive last half-tile index of each group
SBUFS = 17                  # 16 halves per max group + 1 spare

B_FULL, N_FULL = 4, 4096
N_CORES = 8

_nc_cache = None


def _raw(inst):
    return getattr(inst, "ins", inst)


def _build():
    """Build the single-core Bass program (identical on all 8 cores)."""
    nc = bacc.Bacc()
    aT_d = nc.declare_dram_parameter("aT", [D, M], F32R, isOutput=False)
    bT_d = nc.declare_dram_parameter("bT", [D, N], F32R, isOutput=False)
    a_d = nc.declare_dram_parameter("a", [M, D], F32, isOutput=False)
    out_d = nc.declare_dram_parameter("o", [M, N], F32, isOutput=True)
    dbg_d = nc.declare_dram_parameter("dbg", [P, N], F32, isOutput=True)

    out_r = out_d[:, :].rearrange("(t p) n -> t p n", p=P)

    with TileContext(nc) as tc:
        with (
            tc.tile_pool(name="const", bufs=1) as const,
            tc.tile_pool(name="psum", bufs=2, space="PSUM") as psum,
        ):
            aT_r = const.tile([P, KS, M], F32R)
            bT_r = const.tile([P, KS, N], F32R)
            na_pm = const.tile([P, MT], F32)      # per-partition na bias
            nbh_repl = const.tile([P, N], F32)    # nb/2 on every partition
            ones2 = const.tile([P, 2], F32R)      # norm-reduce lhsT
            ones_row = const.tile([P, P], F32R)   # outer-product lhsT [1,128]

            # load order matters: bT feeds the longest preamble chain
            nc.sync.dma_start(
                out=bT_r[:], in_=bT_d[:, :].rearrange("(ks p) n -> p ks n", p=P))
            nc.vector.memset(ones2[:, :].bitcast(U32), 0x3F800000)
            nc.vector.memset(ones_row[0:1, :].bitcast(U32), 0x3F800000)

            with tc.tile_pool(name="tmp", bufs=1) as tmp:
                a_rm = tmp.tile([P, MT, D], F32, tag="arm")
                nc.sync.dma_start(
                    out=a_rm[:], in_=a_d[:, :].rearrange("(t p) k -> p t k", p=P))
                nc.sync.dma_start(
                    out=aT_r[:], in_=aT_d[:, :].rearrange("(ks p) m -> p ks m", p=P))

                # --- na (per-partition, fp32): square + innermost reduce
                asq = tmp.tile([P, MT, D], F32, tag="asq")
                nc.vector.tensor_tensor(
                    out=asq[:], in0=a_rm[:], in1=a_rm[:], op=AL.mult)
                nc.vector.reduce_sum(na_pm[:, :, None], asq[:], axis=AX.X)

                # --- nb/2 via f32r ones-matmul, pipelined per half so the
                # main loop's first units unblock as early as possible
                bsq = tmp.tile([P, KS, N], F32R, tag="bsq")
                nbh_row = tmp.tile([P, N], F32, tag="nbrow")
                for half in range(N // PSW):
                    hsl = slice(half * PSW, (half + 1) * PSW)
                    nc.vector.tensor_tensor(
                        out=bsq[:, :, hsl], in0=bT_r[:, :, hsl],
                        in1=bT_r[:, :, hsl], op=AL.mult)
                    pnb = psum.tile([P, PSW], F32, tag="pt")
                    for c in range(PSW // NCH):
                        n0 = half * PSW + c * NCH
                        for k in range(KS):
                            nc.tensor.matmul(
                                pnb[0:2, c * NCH:(c + 1) * NCH],
                                lhsT=ones2[:, :],
                                rhs=bsq[:, k, n0:n0 + NCH],
                                start=(k == 0),
                                stop=(k == KS - 1),
                            )
                    nc.vector.tensor_scalar_mul(nbh_row[0:1, hsl],
                                                pnb[0:1, :], 0.5)
                    # replicate partition 0 to all partitions (fp32, GpSimd)
                    nc.gpsimd.partition_broadcast(
                        nbh_repl[:, hsl], nbh_row[0:1, hsl])

            # --- main loop (half-tile units: u = (i, half)) ---
            with tc.tile_pool(name="stage", bufs=SBUFS) as stage_pool:
                staged = []
                last_exp = [None]

                def flush():
                    first_exp = None
                    for st_, i_, half_, last_sqrt in staged:
                        e = nc.scalar.activation(
                            out=st_[:], in_=st_[:], func=AF.Exp,
                            bias=0.0, scale=-1.0,
                        )
                        if first_exp is None:
                            first_exp = e
                            # no exp before the group's last sqrt
                            add_dep_helper(_raw(e), _raw(last_sqrt),
                                           reason="act-table group: exp after sqrt")
                        last_exp[0] = e
                        nc.sync.dma_start(
                            out=out_r[i_, :, half_ * PSW:(half_ + 1) * PSW],
                            in_=st_[:])
                    staged.clear()

                for u in range(MT * 2):
                    i, half = divmod(u, 2)
                    st = stage_pool.tile([P, PSW], F32, tag="stage")
                    m0 = i * P
                    pt = psum.tile([P, PSW], F32, tag="pt")
                    for c in range(PSW // NCH):
                        n0 = half * PSW + c * NCH
                        ps = pt[:, c * NCH:(c + 1) * NCH]
                        for k in range(KS):
                            nc.tensor.matmul(
                                ps,
                                lhsT=aT_r[:, k, m0:m0 + P],
                                rhs=bT_r[:, k, n0:n0 + NCH],
                                start=(k == 0),
                                stop=(k == KS - 1),
                            )
                    # u = a.b - nb/2  (PSUM -> SBUF staging)
                    nc.vector.tensor_tensor(
                        out=st[:], in0=pt[:],
                        in1=nbh_repl[:, half * PSW:(half + 1) * PSW],
                        op=AL.subtract,
                    )
                    # D = sqrt(-2*u + na)
                    s = nc.scalar.activation(
                        out=st[:], in_=st[:], func=AF.Sqrt,
                        bias=na_pm[:, i:i + 1], scale=-2.0,
                    )
                    if last_exp[0] is not None:
                        # no sqrt of this group before last group's exps
                        add_dep_helper(_raw(s), _raw(last_exp[0]),
                                       reason="act-table group: sqrt after exp")
                        last_exp[0] = None
                    if i == 0:
                        # debug tap: D values of the first row-tile
                        nc.sync.dma_start(
                            out=dbg_d[:, half * PSW:(half + 1) * PSW],
                            in_=st[:])
                    staged.append((st, i, half, s))
                    if u in GROUP_ENDS:
                        flush()

    nc.compile()
    return nc


def _get_nc():
    global _nc_cache
    if _nc_cache is None:
        _nc_cache = _build()
    return _nc_cache


def _make_in_maps(inputs1, inputs2):
    inputs1 = np.asarray(inputs1, dtype=np.float32)
    inputs2 = np.asarray(inputs2, dtype=np.float32)
    in_maps = []
    for c in range(N_CORES):
        b, h = divmod(c, 2)
        a = inputs1[b, h * M:(h + 1) * M, :]
        in_maps.append({
            "aT": np.ascontiguousarray(a.T),
            "bT": np.ascontiguousarray(inputs2[b].T),
            "a": np.ascontiguousarray(a),
        })
    return in_maps


def _run_spmd(inputs1, inputs2, trace=False):
    nc = _get_nc()
    in_maps = _make_in_maps(inputs1, inputs2)
    return run_bass_kernel_spmd(nc, in_maps, core_ids=list(range(N_CORES)),
                                trace=trace)


def _assemble(results):
    out = np.empty((B_FULL, 2 * M, N_FULL), np.float32)
    for c in range(N_CORES):
        b, h = divmod(c, 2)
        out[b, h * M:(h + 1) * M, :] = results[c]["o"]
    return out


def kernel(inputs1, inputs2):
    res = _run_spmd(inputs1, inputs2, trace=False)
    return _assemble(res.results)
